# revision 1
# baseline (speedup 1.0000x reference)
"""Trainium2 Bass kernel for nn_CustomMultiHeadAttention_20418274525443.

Self-contained: takes FULL unsharded inputs (as produced by the problem's
setup_inputs), shards across 8 NeuronCores, runs a Bass/Tile kernel via
run_bass_kernel_spmd, and gathers the full output.

Sharding: core c handles batch b = c//4 and heads 4*(c%4) .. 4*(c%4)+3
(data parallel on B x tensor parallel on heads). Each core computes its
partial output projection (contribution of its 256 hidden dims); the host
sums the 4 partials per batch and adds the output bias.

Stoichiometric bias via matmul-fused polynomial: the pairwise bias
  bias(f_q, f_k) = alpha * copysign(log1p|f_q - f_k|, f_q - f_k)
is a smooth odd function of d = f_q - f_k; we approximate alpha*g(d) by an
odd polynomial sum_k c_k d^k (degrees 1..9, max err ~2.6e-3 on g) and
expand in centered variables u = f_q - 1/2, v = f_k - 1/2:
  8*alpha*sum_k c_k (u-v)^k = sum_{j=0..9} uq_j(u) * vk_j(v)
The 10 rank terms become 10 extra contraction rows appended to each head's
Q^T/K^T tiles (head_dim 64 -> K=74 matmul): matmul cost is N-cycles
regardless of K<=128, so the bias costs ZERO extra PE cycles and removes
the per-head PSUM-preload matmuls, the per-tile log1p/sign vector chain,
and the Ln<->Exp activation-table switches of the previous version.
(The factor 8 pre-compensates the 0.125 softmax scale applied in exp.)

Math per core (Dh = 64, scale = 1/8):
  per head h: tiles qt_h/kt_h [128, T]: rows 0:64 = head's Q^T/K^T,
    rows 64:74 = basis uq/vk (host-computed from stoich_frac).
  scores^T[k,q] PSUM = kt_h[0:74].T @ qt_h[0:74]   (includes 8*bias)
  probs^T = Exp(0.125 * PSUM + kpm_bias_k) -> f16
  outext^T[65, q]: lhsT=[V_h|ones] so row 64 = sum_k probs (denominator)
  scaled^T = outext^T[0:64] * (1 / outext^T[64])
  out_partial[q, :] = concat_h(scaled^T).T @ Wo_slice^T  [2048, 1024] fp32
"""
import sys

sys.path.insert(0, "/opt/trn_rl_repo")

import numpy as np
from math import comb

import concourse.bass as bass
import concourse.tile as tile
from concourse import bacc, mybir
from concourse.bass_utils import run_bass_kernel_spmd
from concourse.masks import make_identity

D_MODEL = 1024
NHEAD = 16
HEAD_DIM = 64
B, T = 2, 2048
N_CORES = 8
HPC = 4               # heads per core
DSL = HPC * HEAD_DIM  # 256 = per-core slice of d_model
SCALE = HEAD_DIM ** -0.5  # 0.125
QH = 2                # q halves
QHW = T // QH         # 1024
KT = T // 128         # 16 k tiles
NB = 10               # stoich-bias basis rank (poly degrees 0..9)
KEXT = HEAD_DIM + NB  # 74 = extended contraction
F32 = mybir.dt.float32
F16 = mybir.dt.float16
BF16 = mybir.dt.bfloat16
AF = mybir.ActivationFunctionType
ALU = mybir.AluOpType

# odd-polynomial coefficients for g(d) = copysign(log1p|d|, d) on [-1, 1]
# (chebyshev-weighted least squares, degrees 1,3,5,7,9; max err 2.6e-3)
_POLY_DEGS = (1, 3, 5, 7, 9)

_PROGRAM_CACHE = {}
_POLY_CACHE = {}


def _poly_coeffs():
    if "c" not in _POLY_CACHE:
        d = np.linspace(-1, 1, 20001)
        g = np.sign(d) * np.log1p(np.abs(d))
        A = np.stack([d ** k for k in _POLY_DEGS], 1)
        w = 1.0 / np.sqrt(1 - np.clip(d, -0.9999, 0.9999) ** 2)
        _POLY_CACHE["c"] = np.linalg.lstsq(A * w[:, None], g * w, rcond=None)[0]
    return _POLY_CACHE["c"]


def _build_basis(f, alpha):
    """uq [NB, T] f16 (q-side), vk [NB, T] f16 (k-side):
    sum_j uq[j, q] * vk[j, k] == 8 * alpha * g_poly(f[q] - f[k])."""
    c = _poly_coeffs()
    u = np.asarray(f, np.float64) - 0.5
    uq = np.stack([u ** j for j in range(NB)], 0)
    vk = np.zeros((NB, u.shape[0]))
    for j in range(NB):
        for k, ck in zip(_POLY_DEGS, c):
            if k >= j:
                vk[j] += 8.0 * alpha * ck * comb(k, j) * ((-u) ** (k - j))
    for j in range(NB):  # balance magnitudes for f16
        su = np.abs(uq[j]).max()
        sv = np.abs(vk[j]).max()
        if su > 0 and sv > 0:
            s = np.sqrt(sv / su)
            uq[j] *= s
            vk[j] /= s
    return uq.astype(np.float16), vk.astype(np.float16)


def _build_program(use_attn_mask: bool, repeat: int = 1, limit: int = 99):
    # limit: 1=consts+weights, 2=+projections, 4=+scores/exp,
    # 5=+PV/normalize, 99=full
    nc = bacc.Bacc(num_devices=N_CORES)

    # ---- per-core DRAM inputs (host pre-sliced / transposed / cast) ----
    xq_t = nc.dram_tensor("xq_t", [D_MODEL, T], F16, kind="ExternalInput")
    xk_t = nc.dram_tensor("xk_t", [D_MODEL, T], F16, kind="ExternalInput")
    xv_t = nc.dram_tensor("xv_t", [D_MODEL, T], F16, kind="ExternalInput")
    wq_t = nc.dram_tensor("wq_t", [D_MODEL, DSL], F16, kind="ExternalInput")
    wk_t = nc.dram_tensor("wk_t", [D_MODEL, DSL], F16, kind="ExternalInput")
    wv_t = nc.dram_tensor("wv_t", [D_MODEL, DSL], F16, kind="ExternalInput")
    wo_t = nc.dram_tensor("wo_t", [DSL, D_MODEL], F16, kind="ExternalInput")
    bqc_d = nc.dram_tensor("bqc_d", [128, 2], F32, kind="ExternalInput")
    bkc_d = nc.dram_tensor("bkc_d", [128, 2], F32, kind="ExternalInput")
    bvb_d = nc.dram_tensor("bvb_d", [128, DSL], F32, kind="ExternalInput")
    kb_d = nc.dram_tensor("kb_d", [128, KT], F32, kind="ExternalInput")
    uq_d = nc.dram_tensor("uq_d", [NB, T], F16, kind="ExternalInput")
    vk_d = nc.dram_tensor("vk_d", [NB, T], F16, kind="ExternalInput")
    if use_attn_mask:
        am_d = nc.dram_tensor("am_d", [T, T], F16, kind="ExternalInput")
    out_d = nc.dram_tensor("out_d", [T, D_MODEL], F16, kind="ExternalOutput")

    def mm(out_ap, lhsT, rhs, start, stop, nmax=512):
        # matmul with the free dim chunked to one PSUM bank (<=512 fp32)
        n = rhs.shape[-1]
        assert out_ap.shape[-1] == n
        for c0 in range(0, n, nmax):
            c = slice(c0, min(c0 + nmax, n))
            nc.tensor.matmul(out_ap[:, c], lhsT, rhs[:, c],
                             start=start, stop=stop)

    with tile.TileContext(nc) as tc:
        import contextlib
        with contextlib.ExitStack() as ctx:
            const = ctx.enter_context(tc.tile_pool(name="const", bufs=1))
            qk_pool = ctx.enter_context(tc.tile_pool(name="qk", bufs=1))
            v_pool = ctx.enter_context(tc.tile_pool(name="vsb", bufs=1))
            w2_pool = ctx.enter_context(tc.tile_pool(name="wo", bufs=1))
            probs_pool = ctx.enter_context(tc.tile_pool(name="probs", bufs=6))
            den_pool = ctx.enter_context(tc.tile_pool(name="den", bufs=2))
            opair_pool = ctx.enter_context(tc.tile_pool(name="opair", bufs=4))
            ostage_pool = ctx.enter_context(tc.tile_pool(name="ostage", bufs=1))

            # ---- weights: one batched strided DMA per tensor; block di
            # lives at cols [di*DSL, (di+1)*DSL) ----
            wbig = {}
            for nm, dram in (("q", wq_t), ("k", wk_t), ("v", wv_t)):
                t_ = const.tile([128, 8 * DSL], F16, name=f"w{nm}", tag=f"w{nm}")
                nc.sync.dma_start(
                    out=t_[:, :].rearrange("p (a d) -> p a d", a=8),
                    in_=dram.ap().rearrange("(a p) d -> p a d", p=128))
                wbig[nm] = t_

            def wsl(nm, di, c0, c1):
                return wbig[nm][:, di * DSL + c0:di * DSL + c1]

            # ---- constants (Act hwdge queue; SP queue stays free for x) ----
            ones64 = const.tile([1, 64], F32)
            nc.vector.memset(ones64[:, :], 1.0)
            kbias = const.tile([128, KT], F32)
            nc.scalar.dma_start(out=kbias[:, :], in_=kb_d.ap())
            bq_col = const.tile([128, 2], F32)
            nc.scalar.dma_start(out=bq_col[:, :], in_=bqc_d.ap())
            bk_col = const.tile([128, 2], F32)
            nc.scalar.dma_start(out=bk_col[:, :], in_=bkc_d.ap())
            bv_bc = const.tile([128, DSL], F32)
            nc.scalar.dma_start(out=bv_bc[:, :], in_=bvb_d.ap())
            if use_attn_mask:
                ident_f = const.tile([128, 128], F32)
                make_identity(nc, ident_f[:, :])
                ident8 = const.tile([128, 128], F16)
                nc.vector.tensor_scalar(ident8[:, :], ident_f[:, :], 8.0, None,
                                        op0=ALU.mult)
            wo_sb = []
            for pr in range(2):
                t_ = w2_pool.tile([128, D_MODEL], F16, name=f"wo{pr}")
                nc.scalar.dma_start(out=t_[:, :],
                                    in_=wo_t.ap()[pr * 128:(pr + 1) * 128, :])
                wo_sb.append(t_)

            if use_attn_mask:
                am_sb = []
                for kt in range(KT):
                    t_ = const.tile([128, T], F16, name=f"am{kt}", tag=f"am{kt}")
                    # host passes attn_mask^T, [k, q] orientation (x8 via ident8)
                    nc.scalar.dma_start(out=t_[:, :],
                                        in_=am_d.ap()[kt * 128:(kt + 1) * 128, :])
                    am_sb.append(t_)

            # ---- per-head extended Q/K tiles + V tiles (buffers persist
            # across reps; data rows rewritten per rep, basis rows and the
            # ones-columns of V written once) ----
            qt_sb, kt_sb = [], []
            for h in range(HPC):
                qt_ = qk_pool.tile([128, T], F16, name=f"qth{h}")
                nc.scalar.dma_start(out=qt_[HEAD_DIM:KEXT, :], in_=uq_d.ap())
                qt_sb.append(qt_)
                kt_ = qk_pool.tile([128, T], F16, name=f"kth{h}")
                nc.scalar.dma_start(out=kt_[HEAD_DIM:KEXT, :], in_=vk_d.ap())
                kt_sb.append(kt_)
            v_sb = []
            for kt in range(KT):
                t_ = v_pool.tile([128, HPC * 65], F16, name=f"v{kt}")
                nc.vector.memset(t_[:, :], 1.0)  # ones columns survive at 65h+64
                v_sb.append(t_)

            for _rep in range(repeat):
                # ---- phase 1: projections ----
                if limit < 2:
                    continue
                xt_ctx = tc.tile_pool(name="xt", bufs=16)
                xt_pool = xt_ctx.__enter__()
                try:
                    # all x DMAs up front, alternating the two hwdge queues;
                    # they stream in arrival order while the PE computes
                    x_t = {}
                    for nm, xdram in (("q", xq_t), ("k", xk_t), ("v", xv_t)):
                        for di in range(8):
                            xt_ = xt_pool.tile([128, T], F16, name=f"x{nm}{di}",
                                               tag="xt")
                            eng = nc.sync if di % 2 == 0 else nc.scalar
                            eng.dma_start(
                                out=xt_[:, :],
                                in_=xdram.ap()[di * 128:(di + 1) * 128, :])
                            x_t[nm, di] = xt_
                    # Q/K projections: two di-outer passes of 2 groups each
                    # (psA = 4 PSUM banks, freed before phase 2 so the psS
                    # pool starts without waiting on V)
                    with tc.tile_pool(name="psA", bufs=2, space="PSUM") as psA:
                        for nm, bias_col, outs in (("q", bq_col, qt_sb),
                                                   ("k", bk_col, kt_sb)):
                            for do_t in range(2):
                                grp = {}
                                for nch in range(2):
                                    grp[nch] = psA.tile(
                                        [128, QHW], F32,
                                        name=f"psA{nch}", tag="psA")
                                for di in range(8):
                                    for nch in range(2):
                                        mm(grp[nch][:, :],
                                           wsl(nm, di, do_t * 128,
                                               (do_t + 1) * 128),
                                           x_t[nm, di][:, nch * QHW:
                                                       (nch + 1) * QHW],
                                           start=(di == 0), stop=(di == 7))
                                # PSUM -> per-head SBUF f16 with bias add;
                                # psum rows 64:128 shift to head-tile rows
                                # 0:64 (DVE); unshifted rows go via Act Copy
                                # (same act table as Exp, so no table load)
                                for nch in range(2):
                                    nsl = slice(nch * QHW, (nch + 1) * QHW)
                                    h0 = 2 * do_t
                                    nc.scalar.activation(
                                        outs[h0][0:HEAD_DIM, nsl],
                                        grp[nch][0:64, :], AF.Identity,
                                        bias=bias_col[0:64, do_t:do_t + 1],
                                        scale=1.0)
                                    nc.vector.tensor_scalar(
                                        outs[h0 + 1][0:HEAD_DIM, nsl],
                                        grp[nch][64:128, :],
                                        bias_col[64:128, do_t:do_t + 1],
                                        None, op0=ALU.add)

                    # ---- phase 2: attention + output proj, per q-half ----
                # PSUM budget (8 banks): psS 2 x [128,1024] = 4 banks,
                # psO 2 x [65,1024] = 4 banks.  Heads processed in pairs,
                # interleaved per k-tile so the Act engine (exp) never
                # starves; PV lags QK/exp by one k-tile.
                    if limit < 4:
                        continue
                    psS_ctx = tc.tile_pool(name="psS", bufs=2, space="PSUM")
                    psO_ctx = tc.tile_pool(name="psO", bufs=2, space="PSUM")
                    psS = psS_ctx.__enter__()
                    psO = psO_ctx.__enter__()
                    from collections import deque
                    pending = deque()

                    def drain(n=1):
                        for _ in range(n):
                            if pending:
                                pending.popleft()()

                    def v_emit(tt):
                        # V projection k-tile (natural layout), deferred into
                        # pair-0's loop; borrows a psS slot (one accumulation
                        # group in the slot's first bank)
                        ps = psS.tile([128, QHW], F32, name="psv", tag="psS")
                        for di in range(8):
                            mm(ps[:, 0:DSL],
                               x_t["v", di][:, tt * 128:(tt + 1) * 128],
                               wsl("v", di, 0, DSL),
                               start=(di == 0), stop=(di == 7))
                        # strided copy into cols h*65..h*65+63 with bv add;
                        # ones columns at h*65+64 from the memset
                        vdst = v_sb[tt][:, :].rearrange(
                            "p (h e) -> p h e", e=65)[:, :, 0:HEAD_DIM]
                        nc.vector.tensor_tensor(
                            vdst,
                            ps[:, 0:DSL].rearrange("p (h e) -> p h e",
                                                   e=HEAD_DIM),
                            bv_bc[:, :].rearrange("p (h e) -> p h e",
                                                  e=HEAD_DIM),
                            op=ALU.add)

                    for tt in range(KT):
                        pending.append(lambda tt=tt: v_emit(tt))

                    def chain_emit(ot, op_t, hh, tail=False):
                        # normalize rows by the sums row (row 64)
                        rc1 = den_pool.tile([1, QHW], F32, name="rc1",
                                            tag="rc1")
                        nc.vector.reciprocal(rc1[:, :], ot[64:65, :])
                        rb = psS.tile([64, QHW], F32, tag="psS")
                        for nch2 in range(2):
                            nc.tensor.matmul(
                                rb[:, nch2 * 512:(nch2 + 1) * 512],
                                ones64[:, :],
                                rc1[:, nch2 * 512:(nch2 + 1) * 512],
                                start=True, stop=True)
                        rec = den_pool.tile([64, QHW], F32, name="rec",
                                            tag="rec")
                        if tail:
                            nc.scalar.activation(rec[:, :], rb[:, :], AF.Copy,
                                                 bias=0.0, scale=1.0)
                        else:
                            nc.vector.tensor_copy(rec[:, :], rb[:, :])
                        nc.vector.tensor_tensor(
                            op_t[hh * 64:(hh + 1) * 64, :],
                            ot[0:64, :], rec[:, :], op=ALU.mult)

                    megas = {}

                    def oproj_emit(qh, q_t):
                        # output projection for column-tile q_t of q-half qh,
                        # staged into the q-half's mega tile
                        if qh not in megas:
                            megas[qh] = ostage_pool.tile(
                                [128, 8 * D_MODEL], F16, name=f"mega{qh}",
                                tag="mega")
                        mega = megas[qh]
                        for nch in range(2):
                            ps = psS.tile([128, 512], F32, tag="psS")
                            for pr_i in range(2):
                                nc.tensor.matmul(
                                    ps[:, :],
                                    opair_qh[qh][pr_i][:,
                                                       q_t * 128:(q_t + 1) * 128],
                                    wo_sb[pr_i][:, nch * 512:(nch + 1) * 512],
                                    start=(pr_i == 0), stop=(pr_i == 1))
                            dst = mega[:, q_t * D_MODEL + nch * 512:
                                       q_t * D_MODEL + (nch + 1) * 512]
                            if qh == 1:
                                nc.scalar.activation(dst, ps[:, :], AF.Copy,
                                                     bias=0.0, scale=1.0)
                            else:
                                nc.vector.tensor_copy(dst, ps[:, :])

                    def oproj_flush(qh):
                        # one strided DMA for the whole q-half
                        nc.sync.dma_start(
                            out=out_d.ap()[qh * QHW:(qh + 1) * QHW, :]
                                .rearrange("(a p) d -> p a d", p=128),
                            in_=megas[qh][:, :]
                                .rearrange("p (a d) -> p a d", a=8))

                    opair_qh = {}
                    for qh in range(QH):
                        qsl = slice(qh * QHW, (qh + 1) * QHW)
                        opairs = []
                        opair_qh[qh] = opairs
                        for pr_i in range(2):
                            hA, hB = 2 * pr_i, 2 * pr_i + 1
                            op_t = opair_pool.tile([128, QHW], F16,
                                                   name=f"opair{qh}_{pr_i}",
                                                   tag="opair")
                            opairs.append(op_t)
                            otA = psO.tile([65, QHW], F32, tag="psO")
                            otB = psO.tile([65, QHW], F32, tag="psO")
                            prs = {}
                            for kt in range(KT):
                                for h, ot in ((hA, otA), (hB, otB)):
                                    sc = psS.tile([128, QHW], F32, tag="psS")
                                    mm(sc[:, :],
                                       kt_sb[h][0:KEXT,
                                                kt * 128:(kt + 1) * 128],
                                       qt_sb[h][0:KEXT, qsl],
                                       start=True, stop=not use_attn_mask)
                                    if use_attn_mask:
                                        mm(sc[:, :], ident8[:, :],
                                           am_sb[kt][:, qsl],
                                           start=False, stop=True)
                                    pr = probs_pool.tile([128, QHW], F16,
                                                         name="pr", tag="pr")
                                    nc.scalar.activation(
                                        pr[:, :], sc[:, :], AF.Exp,
                                        bias=kbias[:, kt:kt + 1], scale=SCALE)
                                    prs[h, kt] = pr
                                if limit >= 5 and kt > 1:
                                    for h, ot in ((hA, otA), (hB, otB)):
                                        mm(ot[:, :],
                                           v_sb[kt - 2][:,
                                                        h * 65:(h + 1) * 65],
                                           prs[h, kt - 2][:, :],
                                           start=(kt == 2), stop=False)
                                        del prs[h, kt - 2]
                                # drain one deferred V-proj / denominator /
                                # O-proj item per k-tile iteration
                                drain(1)
                            if limit < 5:
                                continue
                            for ktt in (KT - 2, KT - 1):
                                for h, ot in ((hA, otA), (hB, otB)):
                                    mm(ot[:, :],
                                       v_sb[ktt][:, h * 65:(h + 1) * 65],
                                       prs[h, ktt][:, :],
                                       start=False, stop=(ktt == KT - 1))
                            last = (qh == 1 and pr_i == 1)
                            pending.append(
                                lambda ot=otA, op=op_t, tl=last:
                                chain_emit(ot, op, 0, tl))
                            pending.append(
                                lambda ot=otB, op=op_t, tl=last:
                                chain_emit(ot, op, 1, tl))
                        if limit >= 6:
                            for q_t in range(QHW // 128):
                                pending.append(
                                    lambda qh=qh, q_t=q_t: oproj_emit(qh, q_t))
                            pending.append(lambda qh=qh: oproj_flush(qh))
                    drain(len(pending))
                    psO_ctx.__exit__(None, None, None)
                    psS_ctx.__exit__(None, None, None)
                finally:
                    xt_ctx.__exit__(None, None, None)

    nc.compile()
    return nc


def _get_program(use_attn_mask: bool, repeat: int = 1, limit: int = 99):
    key = (use_attn_mask, repeat, limit)
    if key not in _PROGRAM_CACHE:
        _PROGRAM_CACHE[key] = _build_program(use_attn_mask, repeat, limit)
    return _PROGRAM_CACHE[key]


def _prep_in_maps(query, key, value, key_padding_mask, attn_mask, stoich_frac,
                  Wq, bq, Wk, bk, Wv, bv, Wo, stoich_alpha, use_attn_mask):
    bf = np.float16
    f16 = np.float16
    alpha = float(stoich_alpha)
    xt = {}
    for b in range(B):
        xt["q", b] = np.ascontiguousarray(query[b].T).astype(bf)
        xt["k", b] = np.ascontiguousarray(key[b].T).astype(bf)
        xt["v", b] = np.ascontiguousarray(value[b].T).astype(bf)
    uqs, vks, kb = {}, {}, {}
    for b in range(B):
        f32 = np.asarray(stoich_frac[b], np.float32)
        uqs[b], vks[b] = _build_basis(f32, alpha)
        kbv = -30000.0 * np.asarray(key_padding_mask[b], np.float32)
        kb[b] = np.ascontiguousarray(kbv.reshape(KT, 128).T)
    wqT = np.ascontiguousarray(Wq.T).astype(bf)
    wkT = np.ascontiguousarray(Wk.T).astype(bf)
    wvT = np.ascontiguousarray(Wv.T).astype(bf)
    if use_attn_mask:
        am8t = np.ascontiguousarray(attn_mask.T).astype(f16)
    in_maps = []
    for c in range(N_CORES):
        b = c // 4
        g = c % 4
        sl = slice(g * DSL, (g + 1) * DSL)
        m = {
            "xq_t": xt["q", b],
            "xk_t": xt["k", b],
            "xv_t": xt["v", b],
            "wq_t": np.ascontiguousarray(wqT[:, sl]),
            "wk_t": np.ascontiguousarray(wkT[:, sl]),
            "wv_t": np.ascontiguousarray(wvT[:, sl]),
            "wo_t": np.ascontiguousarray(Wo[:, sl].T).astype(bf),
            "bqc_d": np.ascontiguousarray(
                np.asarray(bq[sl], np.float32).reshape(2, 128).T),
            "bkc_d": np.ascontiguousarray(
                np.asarray(bk[sl], np.float32).reshape(2, 128).T),
            "bvb_d": np.ascontiguousarray(np.broadcast_to(
                np.asarray(bv[sl], np.float32), (128, DSL))),
            "kb_d": kb[b],
            "uq_d": uqs[b],
            "vk_d": vks[b],
        }
        if use_attn_mask:
            m["am_d"] = am8t
        in_maps.append(m)
    return in_maps


def kernel(query, key, value, key_padding_mask, attn_mask, stoich_frac,
           Wq, bq, Wk, bk, Wv, bv, Wo, bo, stoich_alpha):
    query = np.asarray(query, np.float32)
    key = np.asarray(key, np.float32)
    value = np.asarray(value, np.float32)
    key_padding_mask = np.asarray(key_padding_mask)
    attn_mask = np.asarray(attn_mask, np.float32)
    stoich_frac = np.asarray(stoich_frac, np.float32)
    use_attn_mask = bool(np.any(attn_mask))

    nc = _get_program(use_attn_mask)
    in_maps = _prep_in_maps(query, key, value, key_padding_mask, attn_mask,
                            stoich_frac, Wq, bq, Wk, bk, Wv, bv, Wo,
                            stoich_alpha, use_attn_mask)
    res = run_bass_kernel_spmd(nc, in_maps, core_ids=list(range(N_CORES)))

    out = np.zeros((B, T, D_MODEL), np.float32)
    for c in range(N_CORES):
        out[c // 4] += np.asarray(res.results[c]["out_d"], np.float32)
    out += np.asarray(bo, np.float32)[None, None, :]
    return out

